# revision 14
# baseline (speedup 1.0000x reference)
"""Multi-head attention (B=2, S=2048, E=1024, H=16, D=64) on 8 Trainium2 NeuronCores.

Sharding: core c -> batch b=c//4, head-group g=c%4 (4 heads per core). Each core
runs the full per-group attention pipeline; the host sums the 4 partial
out-projections per batch element.

v5 kernel: ACT-saturated software pipeline.
  HW facts (microbenchmarked): score matmul pairs (K=64, lhsT bases 0/64)
  auto-derive tile_position (0,0)/(64,0) and run CONCURRENTLY (~95ns/MM);
  N=512 bf16 matmuls stream at ~205-214ns with LDWEIGHTS hidden; an ACT exp
  of a [128,1024] PSUM tile costs (1024+352)/1.2GHz ~= 1.15us. Totals: PE
  ~135us, ACT ~147us -> ACT (exp) is the bottleneck, so the kernel keeps ACT
  back-to-back and hides ALL other PE work inside the exp stream:
  - 8 attention blocks (pair p, q-block qb) x 16 kt steps: score-pair -> exp;
    po accumulation lags LAG kt steps so the PE's strict in-order queue never
    blocks the next scores behind an exp wait.
  - Projections and out-projections ride inside the blocks as interleaved
    FILLER PIECES of <=2 matmuls (a whole 8-MM chain between two score pairs
    would stall the next exp by ~0.7us). Fillers use their own PSUM tag "r"
    (2 bufs) so they never steal the score ring "s" (2 bufs); po uses the
    remaining 2 banks: 4+2+2 = 8 PSUM banks exactly.
  - Queue split: x loads ride the idle HWDGE (sync) queue, den-broadcast
    bounces the pool queue, y stores the gpsimd queue -- so the normalization
    bounce is never stuck behind 256KB y transfers.
  - Block tails on DVE (NOT ACT): reciprocal of the ones-row denominator,
    po->SBUF bf16 copy, DRAM-bounce broadcast of the reciprocal row, then the
    normalization mul.
  Biases are folded into the PSUM->SBUF epilogues (DVE), costing zero PE/ACT.
"""

import numpy as np
import concourse.bass as bass
import concourse.tile as tile
from concourse import bacc, mybir
from concourse.bass_utils import run_bass_kernel_spmd

F32 = mybir.dt.float32
BF16 = mybir.dt.bfloat16
NPBF16 = mybir.dt.np(mybir.dt.bfloat16)

S = 2048
E = 1024
HL = 4        # heads per core
D = 64
QB = 512
NQB = S // QB      # 4
NKT = S // 128     # 16
NKC = E // 128     # 8
LAG = 3            # po lags exp by this many kt steps

_CACHE = {}


def _build(repeat=1):
    nc = bacc.Bacc("TRN2", target_bir_lowering=False, debug=False, num_devices=8)

    xT_d = nc.dram_tensor("xT", [E, S], BF16, kind="ExternalInput").ap()
    wqk_d = nc.dram_tensor("wqk", [E, 512], BF16, kind="ExternalInput").ap()
    bqk_d = nc.dram_tensor("bqk", [128, 4], F32, kind="ExternalInput").ap()
    wv_d = nc.dram_tensor("wv", [E, 256], BF16, kind="ExternalInput").ap()
    bv_d = nc.dram_tensor("bv", [1, 256], F32, kind="ExternalInput").ap()
    wo_d = nc.dram_tensor("wo", [256, E], BF16, kind="ExternalInput").ap()
    bo_d = nc.dram_tensor("bo", [1, E], F32, kind="ExternalInput").ap()
    y_d = nc.dram_tensor("y", [S, E], BF16, kind="ExternalOutput").ap()
    den_d = nc.dram_tensor("den_scratch", [16, QB], BF16)

    with tile.TileContext(nc) as tc:
        with (
            nc.allow_low_precision(reason="bf16 activations by design; f32 accum in PSUM"),
            tc.tile_pool(name="wpool", bufs=1) as wpool,
            tc.tile_pool(name="big", bufs=1) as big,
            tc.tile_pool(name="espool", bufs=8) as espool,
            tc.tile_pool(name="denpool", bufs=4) as denpool,
            tc.tile_pool(name="araw", bufs=4) as araw,
            tc.tile_pool(name="ypool", bufs=3) as ypool,
            tc.tile_pool(name="ps", bufs=2, space="PSUM") as ps,
            tc.tile_pool(name="pq", bufs=2, space="PSUM") as pq,
        ):
            # ---- persistent weights (outside the repeat loop) ----
            wqk_t = []
            for k in range(NKC):
                w = wpool.tile([128, 512], BF16, name=f"wqk{k}")
                nc.sync.dma_start(w[:], wqk_d[k * 128:(k + 1) * 128, :])
                wqk_t.append(w)
            bqk_t = wpool.tile([128, 4], F32, name="bqk_t")
            nc.sync.dma_start(bqk_t[:], bqk_d[:])
            wv_t = []
            for k in range(NKC):
                w = wpool.tile([128, 256], BF16, name=f"wv{k}")
                nc.sync.dma_start(w[:], wv_d[k * 128:(k + 1) * 128, :])
                wv_t.append(w)
            bv_bc = wpool.tile([128, 256], F32, name="bv_bc")
            nc.sync.dma_start(bv_bc[:], bv_d[0:1, :].to_broadcast((128, 256)))
            wo_t = []
            for c in range(2):
                w = wpool.tile([128, E], BF16, name=f"wo{c}")
                nc.sync.dma_start(w[:], wo_d[c * 128:(c + 1) * 128, :])
                wo_t.append(w)
            bo_bc = wpool.tile([128, E], F32, name="bo_bc")
            nc.sync.dma_start(bo_bc[:], bo_d[0:1, :].to_broadcast((128, E)))

            # ACT exp-table warmup: runs during the weight DMAs
            warm = wpool.tile([1, 8], F32, name="warm")
            nc.vector.memset(warm[:], 0.0)
            nc.scalar.activation(warm[:], warm[:], mybir.ActivationFunctionType.Exp)

            # ---- persistent per-iteration tiles ----
            # x is double-buffered: inside the hardware loop a same-buffer
            # reload issues at the body top and its ~12us transfer would be
            # fully exposed (and HAM-cool the PE through the laddered
            # projections). Two buffers + 2x-unrolled loop body let each
            # prefetch hide under the other iteration's attention phase.
            x_t2 = [[big.tile([128, S], BF16, name=f"x{b}_{k}") for k in range(NKC)]
                    for b in range(2)]
            qk_t = [[big.tile([128, QB], BF16, name=f"qk{m}_{qb}") for qb in range(NQB)]
                    for m in range(4)]
            vt = [big.tile([128, HL, 65], BF16, name=f"vt{st}") for st in range(NKT)]
            a_t = [big.tile([128, S], BF16, name=f"a{c}") for c in range(2)]
            # softmax-denominator ones column: written once, never overwritten
            for st in range(NKT):
                nc.vector.memset(vt[st][:, :, 64:65], 1.0)

            def load_x(buf, ks=range(NKC)):
                for k in ks:
                    nc.sync.dma_start(x_t2[buf][k][:], xT_d[k * 128:(k + 1) * 128, :])

            # ---- filler pieces: <=2 matmuls per piece ----
            def qk_pieces(buf, m, qb):
                box = {}
                x_t = x_t2[buf]

                def piece(k0, first, last):
                    def f():
                        if first:
                            box["p"] = ps.tile([128, QB], F32, name="prt", tag="r")
                        p = box["p"]
                        for k in (k0, k0 + 1):
                            nc.tensor.matmul(p[:], wqk_t[k][:, m * 128:(m + 1) * 128],
                                             x_t[k][:, qb * QB:(qb + 1) * QB],
                                             start=(k == 0), stop=(k == NKC - 1))
                        if last:
                            nc.vector.tensor_scalar_add(qk_t[m][qb][:], p[:],
                                                        bqk_t[:, m:m + 1])
                    return f
                return [piece(0, True, False), piece(2, False, False),
                        piece(4, False, False), piece(6, False, True)]

            def qk_proj(buf, m, qb):
                for f in qk_pieces(buf, m, qb):
                    f()

            def v_pieces(buf, st):
                box = {}
                x_t = x_t2[buf]

                def piece(k0, first, last):
                    def f():
                        if first:
                            box["p"] = ps.tile([128, QB], F32, name="prt", tag="r")
                        pv = box["p"][:, 0:256]
                        for k in range(k0, k0 + 4):
                            nc.tensor.matmul(pv, x_t[k][:, st * 128:(st + 1) * 128],
                                             wv_t[k][:], start=(k == 0),
                                             stop=(k == NKC - 1))
                        if last:
                            nc.vector.tensor_add(
                                vt[st][:, :, 0:64],
                                pv.rearrange("p (h d) -> p h d", h=HL),
                                bv_bc[:].rearrange("p (h d) -> p h d", h=HL))
                    return f
                return [piece(0, True, False), piece(4, False, True)]

            def out_pieces(st):
                def piece(n):
                    def f():
                        pt = ps.tile([128, QB], F32, name="prt", tag="r")
                        nc.tensor.matmul(pt[:], a_t[0][:, st * 128:(st + 1) * 128],
                                         wo_t[0][:, n * QB:(n + 1) * QB],
                                         start=True, stop=False)
                        nc.tensor.matmul(pt[:], a_t[1][:, st * 128:(st + 1) * 128],
                                         wo_t[1][:, n * QB:(n + 1) * QB],
                                         start=False, stop=True)
                        yt = ypool.tile([128, QB], BF16, name="yt")
                        nc.vector.tensor_add(yt[:], pt[:], bo_bc[:, n * QB:(n + 1) * QB])
                        nc.gpsimd.dma_start(
                            y_d[st * 128:(st + 1) * 128, n * QB:(n + 1) * QB], yt[:])
                    return f
                return [piece(0), piece(1)]

            def attn_block(pr_i, qb, pieces, offset=0):
                """One (head-pair, q-block) attention block, 16 kt steps.

                Per kt: concurrent score MM pair -> exp (ACT paces the block);
                po MMs run LAG steps behind. `pieces` are small PE filler
                closures spread evenly over kt slots starting at `offset`."""
                j0, j1 = 2 * pr_i, 2 * pr_i + 1
                po_a = pq.tile([65, QB], F32, name="po_a", tag="q")
                po_b = pq.tile([65, QB], F32, name="po_b", tag="q")
                slots = {}
                n = len(pieces)
                for i, f in enumerate(pieces):
                    slots.setdefault(offset + i * (16 - offset) // n, []).append(f)

                def po(kt, es):
                    nc.tensor.matmul(po_a[:], vt[kt][:, j0, :], es[:, 0:QB],
                                     start=(kt == 0), stop=(kt == NKT - 1))
                    nc.tensor.matmul(po_b[:], vt[kt][:, j1, :], es[:, QB:2 * QB],
                                     start=(kt == 0), stop=(kt == NKT - 1))

                pend = []
                for kt in range(NKT):
                    qt_c, cc = kt // 4, (kt % 4) * 128
                    ps_t = ps.tile([128, 2 * QB], F32, name="pst", tag="s")
                    nc.tensor.matmul(ps_t[:, 0:QB], qk_t[2 + pr_i][qt_c][0:64, cc:cc + 128],
                                     qk_t[pr_i][qb][0:64, :], start=True, stop=True)
                    nc.tensor.matmul(ps_t[:, QB:2 * QB],
                                     qk_t[2 + pr_i][qt_c][64:128, cc:cc + 128],
                                     qk_t[pr_i][qb][64:128, :], start=True, stop=True)
                    es = espool.tile([128, 2 * QB], BF16, name="es")
                    nc.scalar.activation(es[:], ps_t[:], mybir.ActivationFunctionType.Exp)
                    pend.append((kt, es))
                    if len(pend) > LAG:
                        po(*pend.pop(0))
                    for f in slots.get(kt, ()):
                        f()
                for kt_es in pend:
                    po(*kt_es)
                # both halves' recip+copy FIRST (frees both po banks and keeps
                # the DVE queue clear), then both muls: a mul waits ~3us on its
                # DRAM-bounce broadcast, and anything queued behind it on DVE
                # would head-block and cascade into the next block's po release.
                # release po FAST: plain tensor_copies only (a DVE reciprocal
                # reading PSUM holds the po bank ~us -> stalls the next block's
                # first po matmul on the pq ring; measured +50us/iter). The
                # reciprocal runs afterwards on the SBUF copy of the den row.
                tail = []
                for half, po_t in ((0, po_a), (1, po_b)):
                    a_raw = araw.tile([64, QB], BF16, name="a_raw")
                    nc.vector.tensor_copy(a_raw[:], po_t[0:64, :])
                    den_row = denpool.tile([1, QB], F32, name="den_row")
                    nc.vector.tensor_copy(den_row[:], po_t[64:65, :])
                    tail.append((half, a_raw, den_row))
                tail2 = []
                for half, a_raw, den_row in tail:
                    den_r = denpool.tile([1, QB], BF16, name="den_r")
                    nc.vector.reciprocal(den_r[:], den_row[:])
                    slot = (2 * pr_i + half) * NQB + qb
                    nc.gpsimd.dma_start(den_d[slot:slot + 1, :], den_r[:])
                    den_sb = denpool.tile([64, QB], BF16, name="den_sb")
                    nc.gpsimd.dma_start(den_sb[:],
                                        den_d[slot:slot + 1, :].to_broadcast((64, QB)))
                    tail2.append((half, a_raw, den_sb))
                # mul on the Pool engine (SBUF-only operands): it waits ~3us on
                # the bounce DMA, and on DVE that wait would head-block the next
                # block's filler epilogues -> "r"-ring stall -> PE stall.
                for half, a_raw, den_sb in tail2:
                    a_out = a_t[pr_i][half * 64:half * 64 + 64, qb * QB:(qb + 1) * QB]
                    nc.gpsimd.tensor_mul(a_out, a_raw[:], den_sb[:])

            def _iter_body(buf, prefetch):
                # dedicated projection phase: on this HW the PE has no slack
                # inside the ACT-paced attention stream, so interleaved filler
                # matmuls cost more (pipeline stalls) than a dense PE-only
                # phase that keeps HAM warm
                for m in (2, 0):
                    for qb in range(NQB):
                        qk_proj(buf, m, qb)
                for st in range(NKT):
                    for f in v_pieces(buf, st):
                        f()
                for m in (3, 1):
                    for qb in range(NQB):
                        qk_proj(buf, m, qb)
                attn_block(0, 0, [])
                attn_block(0, 1, [])
                attn_block(0, 2, [])
                attn_block(0, 3, [])
                # all x readers issued -> prefetch the other x buffer, spread
                # in 2-chunk slices across attention pair 1 so the DMA traffic
                # (shared-LNC bandwidth) never bursts against the exp stream
                if prefetch:
                    load_x(1 - buf, range(0, 2))
                # attention pair 1; fillers: out-projection of finished q-blocks
                attn_block(1, 0, [])
                if prefetch:
                    load_x(1 - buf, range(2, 4))
                attn_block(1, 1, [])
                if prefetch:
                    load_x(1 - buf, range(4, 6))
                attn_block(1, 2, [])
                if prefetch:
                    load_x(1 - buf, range(6, 8))
                attn_block(1, 3, [])
                for st in range(16):
                    for f in out_pieces(st):
                        f()

            load_x(0)
            if repeat == 1:
                _iter_body(0, False)
            else:
                assert repeat % 2 == 0, "repeat must be even (2x-unrolled body)"
                with tc.For_i(0, repeat // 2):
                    _iter_body(0, True)
                    _iter_body(1, True)

    nc.compile()
    return nc


def _shard_inputs(query, W_qkv, b_qkv, W_out, b_out):
    scale = np.float32(1.0 / np.sqrt(D))
    query = np.asarray(query, dtype=np.float32)
    W_qkv = np.asarray(W_qkv, dtype=np.float32)
    b_qkv = np.asarray(b_qkv, dtype=np.float32)
    W_out = np.asarray(W_out, dtype=np.float32)
    b_out = np.asarray(b_out, dtype=np.float32)

    W_q, W_k, W_v = W_qkv[:, :E], W_qkv[:, E:2 * E], W_qkv[:, 2 * E:]
    b_q, b_k, b_v = b_qkv[:E], b_qkv[E:2 * E], b_qkv[2 * E:]

    in_maps = []
    for c in range(8):
        b = c // 4
        g = c % 4
        hsl = slice(4 * g * D, (4 * g + 4) * D)
        wqk = np.empty((E, 512), np.float32)
        wqk[:, :256] = W_q[:, hsl] * scale
        wqk[:, 256:] = W_k[:, hsl]
        bqk_cols = np.empty((512,), np.float32)
        bqk_cols[:256] = b_q[hsl] * scale
        bqk_cols[256:] = b_k[hsl]
        bqk = np.ascontiguousarray(bqk_cols.reshape(4, 128).T)
        in_maps.append({
            "xT": np.ascontiguousarray(query[b].T).astype(NPBF16),
            "wqk": wqk.astype(NPBF16),
            "bqk": bqk,
            "wv": np.ascontiguousarray(W_v[:, hsl]).astype(NPBF16),
            "bv": np.ascontiguousarray(b_v[hsl]).reshape(1, 256),
            "wo": np.ascontiguousarray(W_out[hsl, :]).astype(NPBF16),
            "bo": (b_out if g == 0 else np.zeros_like(b_out)).reshape(1, E),
        })
    return in_maps


def kernel(query, W_qkv, b_qkv, W_out, b_out):
    if "nc" not in _CACHE:
        _CACHE["nc"] = _build()
    nc = _CACHE["nc"]
    in_maps = _shard_inputs(query, W_qkv, b_qkv, W_out, b_out)
    res = run_bass_kernel_spmd(nc, in_maps, list(range(8)))
    out = np.zeros((2, S, E), np.float32)
    for c in range(8):
        out[c // 4] += res.results[c]["y"].astype(np.float32)
    return out
